# revision 1
# baseline (speedup 1.0000x reference)
"""3x3 valid conv (NCHW, stride 1) on 8 Trainium2 NeuronCores.

x: (16, 128, 64, 64) f32, weights: (256, 128, 3, 3) f32
-> out: (16, 256, 62, 62) f32

Data-parallel: 2 images per core, weights replicated. Per core the conv
is 9 shifted accumulated matmuls per output tile: contraction over
cin=128 (SBUF partitions), cout=256 split into two 128-partition PSUM
halves, free dim = 8 output rows x 62 cols = 496 (<= 512 fp32 PSUM bank).
Matmuls run in float32r (full PE rate, ~1e-4 relative error).

Input arrives as 10-row strips (one per 8-row output block) so the first
matmul starts ~2.6us after DMA begins; weights are laid out couth-major
and split into two DMAs for the same reason. Output stores go through
the scalar-engine HWDGE ring so they never queue behind input strips on
the sync ring. Taps iterate outer over quads of 4 row-blocks (4 PSUM
banks each, 8 banks double-buffered) so consecutive matmuls reuse the
same stationary weights where possible.
"""

import numpy as np

N_CORES = 8
IMGS_PER_CORE = 2
CIN = 128
COUT = 256
H = W = 64
OH = OW = 62
RPB = 8  # output rows per block

_NC_CACHE = []


def _build():
    import concourse.bacc as bacc
    import concourse.mybir as mybir
    import concourse.tile as tile

    f32r = mybir.dt.float32r
    f32 = mybir.dt.float32

    nc = bacc.Bacc("TRN2", target_bir_lowering=False, debug=False)
    x = nc.dram_tensor(
        "x", [IMGS_PER_CORE, CIN, H, W], f32r, kind="ExternalInput"
    ).ap()
    # w layout: [cin, (couth, tap, coutl)]; tap = 3*kh + kw
    w = nc.dram_tensor("w", [CIN, 2 * 9 * 128], f32r, kind="ExternalInput").ap()
    out = nc.dram_tensor(
        "out", [IMGS_PER_CORE, COUT, OH, OW], f32, kind="ExternalOutput"
    ).ap()

    with tile.TileContext(nc) as tc:
        with (
            tc.tile_pool(name="wp", bufs=1) as w_pool,
            tc.tile_pool(name="xs", bufs=1) as x_pool,
            tc.tile_pool(name="ost", bufs=6) as out_pool,
            tc.tile_pool(name="ps", bufs=8, space="PSUM") as ps_pool,
        ):
            # PE prewarm: dummy fp32 matmuls on memset scratch keep the PE
            # busy through the HAM window while input DMA streams in, so
            # real matmuls start at 2.4 GHz.
            scr = w_pool.tile([CIN, 128], mybir.dt.float32, tag="scr")
            nc.gpsimd.memset(scr[:], 0.0)
            wp = ps_pool.tile([128, RPB, OW], f32, name="wp", tag="pt")
            for _ in range(12):
                nc.tensor.matmul(
                    wp[:, :2, :], scr[:], scr[:, :124], start=True, stop=True
                )

            w_sb = w_pool.tile([CIN, 2 * 9 * 128], f32r, tag="w")
            strips = {}

            def load_strip(img, b):
                r0 = RPB * b
                nrows = min(RPB + 2, H - r0)  # 10, last block 8
                st = x_pool.tile(
                    [CIN, RPB + 2, W], f32r, name=f"s{img}_{b}", tag=f"s{img}_{b}"
                )
                nc.sync.dma_start(st[:, :nrows, :], x[img, :, r0 : r0 + nrows, :])
                strips[img, b] = st

            # interleave weight halves with the first strips on the sync ring
            nc.sync.dma_start(w_sb[:, :128], w[:, :128])  # h0 tap0
            load_strip(0, 0)
            nc.sync.dma_start(w_sb[:, 128:1152], w[:, 128:1152])  # h0 rest
            load_strip(0, 1)
            nc.sync.dma_start(w_sb[:, 1152:], w[:, 1152:])  # h1
            for b in range(2, 8):
                load_strip(0, b)
            for b in range(8):
                load_strip(1, b)

            first = True
            for img in range(IMGS_PER_CORE):
                for h in range(2):  # cout half
                    # First group runs block-by-block (needs only strip 0 to
                    # start); middle groups run taps outer over all 8 blocks
                    # (8 PSUM banks) so walrus ldw-opt dedupes weight loads;
                    # the final group goes block-by-block again so its
                    # copies/stores drain during compute instead of after
                    # the last matmul.
                    if first:
                        quads = [[0], [1], [2, 3], [4, 5, 6, 7]]
                        first = False
                    elif (img, h) == (IMGS_PER_CORE - 1, 1):
                        quads = [[0, 1, 2, 3], [4, 5], [6], [7]]
                    else:
                        quads = [[0, 1, 2, 3, 4, 5, 6, 7]]
                    for blocks in quads:
                        pts = {}
                        for t in range(9):
                            ki, kj = divmod(t, 3)
                            wsl = w_sb[:, h * 1152 + t * 128 : h * 1152 + t * 128 + 128]
                            for b in blocks:
                                R = min(RPB, OH - RPB * b)  # 8, last block 6
                                if t == 0:
                                    pts[b] = ps_pool.tile(
                                        [128, RPB, OW], f32, name="pt", tag="pt"
                                    )
                                nc.tensor.matmul(
                                    pts[b][:, :R, :],
                                    wsl,
                                    strips[img, b][:, ki : ki + R, kj : kj + OW],
                                    start=(t == 0),
                                    stop=(t == 8),
                                )
                        for b in blocks:
                            R = min(RPB, OH - RPB * b)
                            ot = out_pool.tile([128, RPB, OW], f32)
                            nc.vector.tensor_copy(ot[:, :R, :], pts[b][:, :R, :])
                            nc.scalar.dma_start(
                                out[img, h * 128 : h * 128 + 128, RPB * b : RPB * b + R, :],
                                ot[:, :R, :],
                            )
    nc.compile()
    return nc


def _get_nc():
    if not _NC_CACHE:
        _NC_CACHE.append(_build())
    return _NC_CACHE[0]


def _pack_weights(weights):
    # [cout, cin, kh, kw] -> [cin, couth, kh, kw, coutl] -> [cin, 2*9*128]
    wt = weights.reshape(2, 128, CIN, 3, 3).transpose(2, 0, 3, 4, 1)
    return np.ascontiguousarray(wt.reshape(CIN, 2 * 9 * 128))


def _ldw_opt_patch():
    """Enable walrus's LDWEIGHTS dedup pass (concourse pins it off) so
    back-to-back matmuls sharing a stationary operand skip the reload."""
    import contextlib

    from concourse import bass_utils as _bu

    @contextlib.contextmanager
    def _ctx():
        orig = _bu.run_command

        def patched(argv, **kw):
            argv = [
                "--enable-ldw-opt=true" if a == "--enable-ldw-opt=false" else a
                for a in argv
            ]
            return orig(argv, **kw)

        _bu.run_command = patched
        try:
            yield
        finally:
            _bu.run_command = orig

    return _ctx()


def kernel(x, weights):
    from concourse.bass_utils import run_bass_kernel_spmd

    x = np.ascontiguousarray(x, dtype=np.float32)
    weights = np.ascontiguousarray(weights, dtype=np.float32)
    w_l = _pack_weights(weights)

    nc = _get_nc()
    in_maps = [
        {"x": x[IMGS_PER_CORE * c : IMGS_PER_CORE * (c + 1)], "w": w_l}
        for c in range(N_CORES)
    ]
    with _ldw_opt_patch():
        res = run_bass_kernel_spmd(nc, in_maps, core_ids=list(range(N_CORES)))
    return np.concatenate([r["out"] for r in res.results], axis=0)



# revision 8
# speedup vs baseline: 1.0629x; 1.0629x over previous
"""3x3 valid conv (NCHW, stride 1) on 8 Trainium2 NeuronCores.

x: (16, 128, 64, 64) f32, weights: (256, 128, 3, 3) f32
-> out: (16, 256, 62, 62) f32

Data-parallel: 2 images per core, weights replicated. Per core the conv
is 9 shifted accumulated matmuls per output tile: contraction over
cin=128 (SBUF partitions), cout=256 split into two 128-partition PSUM
halves. Operands are bf16 (host-converted; fp32 PSUM accumulate keeps
rel err ~1e-3, and FWL doubles LDWEIGHTS speed for non-fp32 dtypes).

The moving operand is kept FULLY CONTIGUOUS: input strips live flat in
SBUF ([128, 10*64]) and each matmul streams 512 consecutive elements =
8 full 64-wide input rows (one whole fp32 PSUM bank). Columns 62-63 of
each row accumulate wraparound garbage and are simply never copied out.

Loop order is block-outer / tap-inner: all 9 accumulating matmuls of a
block run back-to-back into the SAME PSUM bank, and the block's
PSUM->SBUF copy + store issue immediately after, so copies/stores drain
evenly through the kernel. LDWEIGHTS (one per matmul) hides under the
previous matmul via the background weight buffer.

Input arrives as 10-row strips (one per 8-row output block), strip 0
first so the PE can start as early as possible; a short prewarm on
uninitialized SBUF keeps the PE busy through the HAM window while that
first strip streams in. Output stores go through the scalar-engine
HWDGE ring; the final block's copy/store is split across scalar+vector
engines and sync+scalar rings to shorten the tail.
"""

import numpy as np

N_CORES = 8
IMGS_PER_CORE = 2
CIN = 128
COUT = 256
H = W = 64
OH = OW = 62
RPB = 8  # output rows per block
STRIP_PAD = 656  # 10*64 rows + slack for the tap-(2,2) overrun
N_PREWARM = 3

_NC_CACHE = []


def _build():
    import concourse.bacc as bacc
    import concourse.mybir as mybir
    import concourse.tile as tile

    bf16 = mybir.dt.bfloat16
    f32 = mybir.dt.float32

    nc = bacc.Bacc("TRN2", target_bir_lowering=False, debug=False)
    x = nc.dram_tensor(
        "x", [IMGS_PER_CORE, CIN, H, W], bf16, kind="ExternalInput"
    ).ap()
    # w layout: [cin, (couth, tap, coutl)]; tap = 3*kh + kw
    w = nc.dram_tensor("w", [CIN, 2 * 9 * 128], bf16, kind="ExternalInput").ap()
    out = nc.dram_tensor(
        "out", [IMGS_PER_CORE, COUT, OH, OW], f32, kind="ExternalOutput"
    ).ap()

    with tile.TileContext(nc) as tc:
        with (
            tc.tile_pool(name="wp", bufs=1) as w_pool,
            tc.tile_pool(name="xs", bufs=1) as x_pool,
            tc.tile_pool(name="ost", bufs=6) as out_pool,
            tc.tile_pool(name="ps", bufs=8, space="PSUM") as ps_pool,
        ):
            # PE prewarm (results discarded; the PSUM banks are fully
            # overwritten by the first start=True real matmul). Keeps the PE
            # busy through the HAM window while the first strip streams in.
            scr_w = w_pool.tile([CIN, 128], bf16, tag="scrw")
            scr_m = w_pool.tile([CIN, 512], bf16, tag="scrm")
            nc.vector.memset(scr_w[:], 0.0)
            nc.vector.memset(scr_m[:], 0.0)
            for _ in range(N_PREWARM):
                wp = ps_pool.tile([128, RPB, 64], f32, name="wp", tag="pt")
                nc.tensor.matmul(wp[:, :, :], scr_w[:], scr_m[:], start=True, stop=True)

            w_sb = w_pool.tile([CIN, 2 * 9 * 128], bf16, tag="w")
            strips = {}

            def load_strip(img, b):
                r0 = RPB * b
                nrows = min(RPB + 2, H - r0)  # 10, last block 8
                st = x_pool.tile(
                    [CIN, STRIP_PAD], bf16, name=f"s{img}_{b}", tag=f"s{img}_{b}"
                )
                nc.sync.dma_start(st[:, : nrows * W], x[img, :, r0 : r0 + nrows, :])
                strips[img, b] = st

            # strip 0 first (gates the first real matmul), then weight
            # chunks interleaved with the remaining strips on the sync ring
            load_strip(0, 0)
            nc.sync.dma_start(w_sb[:, :384], w[:, :384])  # h0 taps 0-2
            load_strip(0, 1)
            nc.sync.dma_start(w_sb[:, 384:1152], w[:, 384:1152])  # h0 taps 3-8
            load_strip(0, 2)
            load_strip(0, 3)
            nc.sync.dma_start(w_sb[:, 1152:], w[:, 1152:])  # h1
            for b in range(4, 8):
                load_strip(0, b)
            for b in range(8):
                load_strip(1, b)

            for img in range(IMGS_PER_CORE):
                for h in range(2):  # cout half
                    last_group = (img, h) == (IMGS_PER_CORE - 1, 1)
                    for b in range(8):
                        R = min(RPB, OH - RPB * b)  # 8, last block 6
                        r0 = RPB * b
                        pt = ps_pool.tile([128, RPB, 64], f32, name="pt", tag="pt")
                        for t in range(9):
                            ki, kj = divmod(t, 3)
                            off = ki * W + kj
                            wsl = w_sb[:, h * 1152 + t * 128 : h * 1152 + t * 128 + 128]
                            nc.tensor.matmul(
                                pt[:, :R, :],
                                wsl,
                                strips[img, b][:, off : off + R * W],
                                start=(t == 0),
                                stop=(t == 8),
                            )
                        ot = out_pool.tile([128, RPB, OW], f32)
                        osl = out[img, h * 128 : h * 128 + 128]
                        if last_group and b == 7:
                            # split the tail copy/store across engines
                            nc.scalar.copy(ot[:, :3, :], pt[:, :3, :OW])
                            nc.vector.tensor_copy(ot[:, 3:R, :], pt[:, 3:R, :OW])
                            nc.sync.dma_start(osl[:, r0 : r0 + 3, :], ot[:, :3, :])
                            nc.scalar.dma_start(
                                osl[:, r0 + 3 : r0 + R, :], ot[:, 3:R, :]
                            )
                        else:
                            nc.vector.tensor_copy(ot[:, :R, :], pt[:, :R, :OW])
                            nc.scalar.dma_start(osl[:, r0 : r0 + R, :], ot[:, :R, :])
    nc.compile()
    return nc


def _get_nc():
    if not _NC_CACHE:
        _NC_CACHE.append(_build())
    return _NC_CACHE[0]


def _pack_weights(weights):
    # [cout, cin, kh, kw] -> [cin, couth, kh, kw, coutl] -> [cin, 2*9*128]
    import ml_dtypes

    wt = weights.reshape(2, 128, CIN, 3, 3).transpose(2, 0, 3, 4, 1)
    return np.ascontiguousarray(
        wt.reshape(CIN, 2 * 9 * 128).astype(ml_dtypes.bfloat16)
    )


def _pack_x(x):
    import ml_dtypes

    return np.ascontiguousarray(x.astype(ml_dtypes.bfloat16))


def _ldw_opt_patch():
    """No-op: with block-outer/tap-inner ordering every matmul changes its
    stationary operand, so walrus's LDWEIGHTS dedup has nothing to dedupe
    (and its codegen rejects bf16/FWL ldweights)."""
    import contextlib

    @contextlib.contextmanager
    def _ctx():
        yield

    return _ctx()


def kernel(x, weights):
    from concourse.bass_utils import run_bass_kernel_spmd

    x = np.ascontiguousarray(x, dtype=np.float32)
    weights = np.ascontiguousarray(weights, dtype=np.float32)
    w_l = _pack_weights(weights)
    x_l = _pack_x(x)

    nc = _get_nc()
    in_maps = [
        {"x": x_l[IMGS_PER_CORE * c : IMGS_PER_CORE * (c + 1)], "w": w_l}
        for c in range(N_CORES)
    ]
    with _ldw_opt_patch():
        res = run_bass_kernel_spmd(nc, in_maps, core_ids=list(range(N_CORES)))
    return np.concatenate([r["out"] for r in res.results], axis=0)


# revision 11
# speedup vs baseline: 1.0771x; 1.0134x over previous
"""3x3 valid conv (NCHW, stride 1) on 8 Trainium2 NeuronCores.

x: (16, 128, 64, 64) f32, weights: (256, 128, 3, 3) f32
-> out: (16, 256, 62, 62) f32

Data-parallel: 2 images per core, weights replicated. Per core the conv
is 9 shifted accumulated matmuls per output tile: contraction over
cin=128 (SBUF partitions), cout=256 split into two 128-partition PSUM
halves. Operands are bf16 (host-converted; fp32 PSUM accumulate keeps
rel err ~1e-3, and FWL doubles LDWEIGHTS speed for non-fp32 dtypes).

The moving operand is kept FULLY CONTIGUOUS: input strips live flat in
SBUF ([128, 10*64]) and each matmul streams 512 consecutive elements =
8 full 64-wide input rows (one whole fp32 PSUM bank). Columns 62-63 of
each row accumulate wraparound garbage and are simply never copied out.

Loop order is block-outer / tap-inner: all 9 accumulating matmuls of a
block run back-to-back into the SAME PSUM bank, and the block's
PSUM->SBUF copy + store issue immediately after, so copies/stores drain
evenly through the kernel. LDWEIGHTS (one per matmul) hides under the
previous matmul via the background weight buffer.

Input arrives as 10-row strips (one per 8-row output block), strip 0
first so the PE can start as early as possible; a short prewarm on
uninitialized SBUF keeps the PE busy through the HAM window while that
first strip streams in. Output stores go through the scalar-engine
HWDGE ring; the final block's copy/store is split across scalar+vector
engines and sync+scalar rings to shorten the tail.
"""

import numpy as np

N_CORES = 8
IMGS_PER_CORE = 2
CIN = 128
COUT = 256
H = W = 64
OH = OW = 62
RPB = 8  # output rows per block
STRIP_PAD = 656  # 10*64 rows + slack for the tap-(2,2) overrun
N_PREWARM = 7

_NC_CACHE = []


def _build():
    import concourse.bacc as bacc
    import concourse.mybir as mybir
    import concourse.tile as tile

    bf16 = mybir.dt.bfloat16
    f32 = mybir.dt.float32

    nc = bacc.Bacc("TRN2", target_bir_lowering=False, debug=False)
    x = nc.dram_tensor(
        "x", [IMGS_PER_CORE, CIN, H, W], bf16, kind="ExternalInput"
    ).ap()
    # w layout: [cin, (couth, tap, coutl)]; tap = 3*kh + kw
    w = nc.dram_tensor("w", [CIN, 2 * 9 * 128], bf16, kind="ExternalInput").ap()
    out = nc.dram_tensor(
        "out", [IMGS_PER_CORE, COUT, OH, OW], f32, kind="ExternalOutput"
    ).ap()

    with tile.TileContext(nc) as tc:
        with (
            tc.tile_pool(name="wp", bufs=1) as w_pool,
            tc.tile_pool(name="xs", bufs=1) as x_pool,
            tc.tile_pool(name="ost", bufs=6) as out_pool,
            tc.tile_pool(name="ps", bufs=8, space="PSUM") as ps_pool,
        ):
            # PE prewarm (results discarded; the PSUM banks are fully
            # overwritten by the first start=True real matmul). Keeps the PE
            # busy through the HAM window while the first strip streams in.
            scr_w = w_pool.tile([CIN, 128], bf16, tag="scrw")
            scr_m = w_pool.tile([CIN, 512], bf16, tag="scrm")
            nc.gpsimd.memset(scr_w[:], 0.0)
            nc.gpsimd.memset(scr_m[:], 0.0)
            for _ in range(N_PREWARM):
                wp = ps_pool.tile([128, RPB, 64], f32, name="wp", tag="pt")
                nc.tensor.matmul(wp[:, :, :], scr_w[:], scr_m[:], start=True, stop=True)

            w_sb = w_pool.tile([CIN, 2 * 9 * 128], bf16, tag="w")
            strips = {}

            def load_strip(img, b):
                r0 = RPB * b
                nrows = min(RPB + 2, H - r0)  # 10, last block 8
                st = x_pool.tile(
                    [CIN, STRIP_PAD], bf16, name=f"s{img}_{b}", tag=f"s{img}_{b}"
                )
                nc.sync.dma_start(st[:, : nrows * W], x[img, :, r0 : r0 + nrows, :])
                strips[img, b] = st

            # strip 0 first (gates the first real matmul), then weight
            # chunks interleaved with the remaining strips on the sync ring
            load_strip(0, 0)
            nc.sync.dma_start(w_sb[:, :384], w[:, :384])  # h0 taps 0-2
            load_strip(0, 1)
            nc.sync.dma_start(w_sb[:, 384:1152], w[:, 384:1152])  # h0 taps 3-8
            load_strip(0, 2)
            load_strip(0, 3)
            nc.sync.dma_start(w_sb[:, 1152:], w[:, 1152:])  # h1
            for b in range(4, 8):
                load_strip(0, b)
            for b in range(8):
                load_strip(1, b)

            for img in range(IMGS_PER_CORE):
                for h in range(2):  # cout half
                    last_group = (img, h) == (IMGS_PER_CORE - 1, 1)
                    for b in range(8):
                        R = min(RPB, OH - RPB * b)  # 8, last block 6
                        r0 = RPB * b
                        pt = ps_pool.tile([128, RPB, 64], f32, name="pt", tag="pt")
                        for t in range(9):
                            ki, kj = divmod(t, 3)
                            off = ki * W + kj
                            wsl = w_sb[:, h * 1152 + t * 128 : h * 1152 + t * 128 + 128]
                            nc.tensor.matmul(
                                pt[:, :R, :],
                                wsl,
                                strips[img, b][:, off : off + R * W],
                                start=(t == 0),
                                stop=(t == 8),
                            )
                        ot = out_pool.tile([128, RPB, OW], f32)
                        osl = out[img, h * 128 : h * 128 + 128]
                        if last_group and b == 6:
                            # scalar-engine copy keeps the vector engine free
                            # for block 7's tail copy
                            nc.scalar.copy(ot[:, :R, :], pt[:, :R, :OW])
                            nc.scalar.dma_start(osl[:, r0 : r0 + R, :], ot[:, :R, :])
                        elif last_group and b == 7:
                            # split the tail copy/store across engines
                            nc.vector.tensor_copy(ot[:, :3, :], pt[:, :3, :OW])
                            nc.scalar.copy(ot[:, 3:R, :], pt[:, 3:R, :OW])
                            nc.sync.dma_start(osl[:, r0 : r0 + 3, :], ot[:, :3, :])
                            nc.scalar.dma_start(
                                osl[:, r0 + 3 : r0 + R, :], ot[:, 3:R, :]
                            )
                        else:
                            nc.vector.tensor_copy(ot[:, :R, :], pt[:, :R, :OW])
                            nc.scalar.dma_start(osl[:, r0 : r0 + R, :], ot[:, :R, :])
    nc.compile()
    return nc


def _get_nc():
    if not _NC_CACHE:
        _NC_CACHE.append(_build())
    return _NC_CACHE[0]


def _pack_weights(weights):
    # [cout, cin, kh, kw] -> [cin, couth, kh, kw, coutl] -> [cin, 2*9*128]
    import ml_dtypes

    wt = weights.reshape(2, 128, CIN, 3, 3).transpose(2, 0, 3, 4, 1)
    return np.ascontiguousarray(
        wt.reshape(CIN, 2 * 9 * 128).astype(ml_dtypes.bfloat16)
    )


def _pack_x(x):
    import ml_dtypes

    return np.ascontiguousarray(x.astype(ml_dtypes.bfloat16))


def _ldw_opt_patch():
    """No-op: with block-outer/tap-inner ordering every matmul changes its
    stationary operand, so walrus's LDWEIGHTS dedup has nothing to dedupe
    (and its codegen rejects bf16/FWL ldweights)."""
    import contextlib

    @contextlib.contextmanager
    def _ctx():
        yield

    return _ctx()


def kernel(x, weights):
    from concourse.bass_utils import run_bass_kernel_spmd

    x = np.ascontiguousarray(x, dtype=np.float32)
    weights = np.ascontiguousarray(weights, dtype=np.float32)
    w_l = _pack_weights(weights)
    x_l = _pack_x(x)

    nc = _get_nc()
    in_maps = [
        {"x": x_l[IMGS_PER_CORE * c : IMGS_PER_CORE * (c + 1)], "w": w_l}
        for c in range(N_CORES)
    ]
    with _ldw_opt_patch():
        res = run_bass_kernel_spmd(nc, in_maps, core_ids=list(range(N_CORES)))
    return np.concatenate([r["out"] for r in res.results], axis=0)
